# revision 9
# baseline (speedup 1.0000x reference)
"""Binarized 3x3 conv (BConv2d) on 8 TRN2 NeuronCores.

Problem: x (32, 32, 256, 256) f32, weight (32, 32, 3, 3) f32.
  out = conv2d(x, sign(weight), padding='same') / sqrt(32*9)

Strategy:
  - Data-parallel over batch: core i gets images 4i..4i+3 (no collectives).
  - Per core, pack 4 images x 32 input channels onto the 128 SBUF
    partitions.  Each 3x3 tap becomes ONE K=128, M=128 matmul with a
    block-diagonal (per-image) binarized weight matrix; the 9 taps
    accumulate into PSUM and differ only in the rhs address offset into a
    zero-padded bf16 copy of the input (258x258 per partition).
  - bf16 inputs (weights are exactly +-1 in bf16), fp32 PSUM accumulate.
  - Pipeline: DMA f32 chunk -> SBUF, ScalarE cast+place into padded bf16
    image, 9x4 matmuls per 8-row super-chunk into 4 PSUM banks, VectorE
    scaled drain to SBUF, DMA out.
"""

import math

import numpy as np
import ml_dtypes

import concourse.bass as bass
import concourse.mybir as mybir
import concourse.tile as tile
from concourse import bacc
from concourse import bass_utils

N_CORES = 8
N_IMG = 4          # images per core
C_IN = 32
C_OUT = 32
K = 3
H = 256
W = 256
DIV = float(np.sqrt(C_IN * K * K))


def build_conv_kernel(
    nimg=N_IMG,
    cin=C_IN,
    cout=C_OUT,
    h=H,
    w=W,
    chunk_rows=8,   # input DMA/cast granularity (rows)
    bank_rows=2,    # output rows per PSUM bank matmul (bank_rows*w <= 512)
    banks_per_sc=4, # PSUM banks per super-chunk
    div=DIV,
    repeats=1,      # execute the whole body N times (for delta-timing)
):
    """Build the per-core Bass graph.  Returns nc (compiled Bacc)."""
    P = nimg * cin
    assert P <= 128
    M = nimg * cout
    assert M <= 128
    assert bank_rows * w <= 512
    sc_rows = bank_rows * banks_per_sc
    assert h % chunk_rows == 0 and h % sc_rows == 0
    hp, wp = h + 2, w + 2
    n_taps = 9

    nc = bacc.Bacc(
        "TRN2", target_bir_lowering=False, debug=False, num_devices=N_CORES
    )
    x_dram = nc.dram_tensor("x", [P, h, w], mybir.dt.float32, kind="ExternalInput")
    w_dram = nc.dram_tensor(
        "w9", [P, n_taps, M], mybir.dt.bfloat16, kind="ExternalInput"
    )
    out_dram = nc.dram_tensor(
        "out", [M, h, w], mybir.dt.float32, kind="ExternalOutput"
    )

    with tile.TileContext(nc) as tc:
        with (
            tc.tile_pool(name="persist", bufs=1) as perpool,
            tc.tile_pool(name="istage", bufs=3) as ipool,
            tc.tile_pool(name="ostage", bufs=3) as opool,
            tc.tile_pool(name="psum", bufs=2 * banks_per_sc, space="PSUM") as ppool,
        ):
            xpad = perpool.tile([P, hp, wp], mybir.dt.bfloat16, name="xpad")
            wsb = perpool.tile([P, n_taps, M], mybir.dt.bfloat16, name="wsb")

            nc.sync.dma_start(out=wsb[:], in_=w_dram[:])
            # zero the padding ring (top/bottom rows, left/right columns)
            nc.vector.memset(xpad[:, 0, :], 0.0)
            nc.vector.memset(xpad[:, hp - 1, :], 0.0)
            nc.vector.memset(xpad[:, :, 0], 0.0)
            nc.vector.memset(xpad[:, :, wp - 1], 0.0)

            def emit_body():
                # input pipeline: DMA f32 chunk, cast to bf16 into padded image
                for c in range(h // chunk_rows):
                    r0 = c * chunk_rows
                    st = ipool.tile([P, chunk_rows, w], mybir.dt.float32, name="st")
                    nc.sync.dma_start(
                        out=st[:], in_=x_dram[:, r0 : r0 + chunk_rows, :]
                    )
                    nc.scalar.copy(
                        out=xpad[:, r0 + 1 : r0 + chunk_rows + 1, 1 : w + 1],
                        in_=st[:],
                    )

                # compute pipeline: super-chunks of sc_rows output rows,
                # one PSUM-bank tile per bank_rows strip (own accum group)
                for s in range(h // sc_rows):
                    h0 = s * sc_rows
                    pts = [
                        ppool.tile(
                            [M, bank_rows, w], mybir.dt.float32,
                            name="pt", tag="pt",
                        )
                        for _ in range(banks_per_sc)
                    ]
                    for t in range(n_taps):
                        dy, dx = t // 3, t % 3
                        lhsT = wsb[:, t, :]
                        for b in range(banks_per_sc):
                            hb = h0 + b * bank_rows
                            nc.tensor.matmul(
                                pts[b][:],
                                lhsT,
                                xpad[:, hb + dy : hb + dy + bank_rows, dx : dx + w],
                                start=(t == 0),
                                stop=(t == n_taps - 1),
                            )
                    ot = opool.tile([M, sc_rows, w], mybir.dt.float32, name="ot")
                    for b in range(banks_per_sc):
                        nc.vector.tensor_scalar_mul(
                            ot[:, b * bank_rows : (b + 1) * bank_rows, :],
                            pts[b][:],
                            1.0 / div,
                        )
                    nc.sync.dma_start(
                        out=out_dram[:, h0 : h0 + sc_rows, :], in_=ot[:]
                    )

            for _rep in range(repeats):
                emit_body()

    nc.compile()
    return nc


def make_weight_tensor(weight, nimg=N_IMG, cin=C_IN, cout=C_OUT):
    """Binarize + block-diagonalize: [cout,cin,3,3] f32 -> [nimg*cin, 9, nimg*cout] bf16."""
    n_taps = weight.shape[2] * weight.shape[3]
    wbin = np.where(weight > 0, 1.0, -1.0).astype(np.float32)
    # [co, ci, kh, kw] -> [ci, t, co]
    wt = wbin.reshape(cout, cin, n_taps).transpose(1, 2, 0)
    w9 = np.zeros((nimg * cin, n_taps, nimg * cout), dtype=ml_dtypes.bfloat16)
    for i in range(nimg):
        w9[i * cin : (i + 1) * cin, :, i * cout : (i + 1) * cout] = wt
    return w9


def kernel(x, weight, trace=False, repeats=1, _nc_cache={}):
    """Full-input entry point: x (32,32,256,256) f32, weight (32,32,3,3) f32."""
    x = np.ascontiguousarray(np.asarray(x, dtype=np.float32))
    weight = np.asarray(weight, dtype=np.float32)
    n_batch = x.shape[0]
    per_core = n_batch // N_CORES

    if repeats not in _nc_cache:
        _nc_cache[repeats] = build_conv_kernel(repeats=repeats)
    nc = _nc_cache[repeats]

    w9 = make_weight_tensor(weight)
    P = N_IMG * C_IN
    in_maps = [
        {
            "x": x[i * per_core : (i + 1) * per_core].reshape(P, H, W),
            "w9": w9,
        }
        for i in range(N_CORES)
    ]
    try:
        res = bass_utils.run_bass_kernel_spmd(
            nc, in_maps, core_ids=list(range(N_CORES)), trace=trace
        )
    except ModuleNotFoundError:
        # axon NTFF profiling hook unavailable in this environment
        res = bass_utils.run_bass_kernel_spmd(
            nc, in_maps, core_ids=list(range(N_CORES)), trace=False
        )
    out = np.concatenate(
        [r["out"].reshape(per_core, C_OUT, H, W) for r in res.results], axis=0
    )
    if trace:
        kernel.last_results = res
    return out


# revision 15
# speedup vs baseline: 4715.0254x; 4715.0254x over previous
"""Binarized 3x3 conv (BConv2d) on 8 TRN2 NeuronCores.

Problem: x (32, 32, 256, 256) f32, weight (32, 32, 3, 3) f32.
  out = conv2d(x, sign(weight), padding='same') / sqrt(32*9)

Strategy:
  - Data-parallel over batch: core i gets images 4i..4i+3 (no collectives).
  - Per core, pack 4 images x 32 input channels onto the 128 SBUF
    partitions.  Each 3x3 tap becomes ONE K=128, M=128 matmul with a
    block-diagonal (per-image) binarized weight matrix; the 9 taps
    accumulate into PSUM and differ only in the rhs address offset into a
    zero-padded bf16 copy of the input (258x258 per partition).
  - bf16 inputs (weights are exactly +-1 in bf16), fp32 PSUM accumulate.
  - Pipeline: DMA f32 chunk -> SBUF, ScalarE cast+place into padded bf16
    image, 9x4 matmuls per 8-row super-chunk into 4 PSUM banks, VectorE
    scaled drain to SBUF, DMA out.
"""

import numpy as np
import ml_dtypes

import concourse.mybir as mybir
import concourse.tile as tile
from concourse import bacc
from concourse import bass_utils

N_CORES = 8
N_IMG = 4          # images per core
C_IN = 32
C_OUT = 32
K = 3
H = 256
W = 256
DIV = float(np.sqrt(C_IN * K * K))


def build_conv_kernel(
    nimg=N_IMG,
    cin=C_IN,
    cout=C_OUT,
    h=H,
    w=W,
    chunk_rows=8,   # input DMA/cast granularity (rows)
    bank_rows=2,    # output rows per PSUM bank matmul (bank_rows*w <= 512)
    banks_per_sc=4, # PSUM banks per super-chunk
    div=DIV,
    repeats=1,      # execute the whole body N times (for delta-timing)
):
    """Build the per-core Bass graph.  Returns nc (compiled Bacc)."""
    P = nimg * cin
    assert P <= 128
    M = nimg * cout
    assert M <= 128
    assert bank_rows * w <= 512
    sc_rows = bank_rows * banks_per_sc
    assert h % chunk_rows == 0 and h % sc_rows == 0
    hp, wp = h + 2, w + 2
    n_taps = 9

    nc = bacc.Bacc(
        "TRN2", target_bir_lowering=False, debug=False, num_devices=N_CORES
    )
    x_dram = nc.dram_tensor("x", [P, h, w], mybir.dt.float32, kind="ExternalInput")
    w_dram = nc.dram_tensor(
        "w9", [P, n_taps, M], mybir.dt.bfloat16, kind="ExternalInput"
    )
    out_dram = nc.dram_tensor(
        "out", [M, h, w], mybir.dt.float32, kind="ExternalOutput"
    )

    with tile.TileContext(nc) as tc:
        with (
            tc.tile_pool(name="persist", bufs=1) as perpool,
            tc.tile_pool(name="istage", bufs=3) as ipool,
            tc.tile_pool(name="ostage", bufs=3) as opool,
            tc.tile_pool(name="psum", bufs=2 * banks_per_sc, space="PSUM") as ppool,
        ):
            xpad = perpool.tile([P, hp, wp], mybir.dt.bfloat16, name="xpad")
            wsb = perpool.tile([P, n_taps, M], mybir.dt.bfloat16, name="wsb")

            nc.sync.dma_start(out=wsb[:], in_=w_dram[:])
            # zero the padding ring (top/bottom rows, left/right columns)
            nc.vector.memset(xpad[:, 0, :], 0.0)
            nc.vector.memset(xpad[:, hp - 1, :], 0.0)
            nc.vector.memset(xpad[:, :, 0], 0.0)
            nc.vector.memset(xpad[:, :, wp - 1], 0.0)

            def emit_input_rows(r0, nrows):
                st = ipool.tile(
                    [P, chunk_rows, w], mybir.dt.float32, name="st", tag="st"
                )
                nc.sync.dma_start(
                    out=st[:, :nrows, :], in_=x_dram[:, r0 : r0 + nrows, :]
                )
                nc.scalar.copy(
                    out=xpad[:, r0 + 1 : r0 + nrows + 1, 1 : w + 1],
                    in_=st[:, :nrows, :],
                )

            def emit_body():
                # Interleave input chunks with compute super-chunks so DMA
                # lane semaphore windows complete progressively (an
                # up-front input burst couples early sem resets to the
                # last input DMA and stalls the whole pipeline mid-kernel).
                next_row = [0]

                def load_until(row_needed):
                    while next_row[0] < min(row_needed, h):
                        r0 = next_row[0]
                        # small first pieces so the first matmuls start early
                        nrows = 2 if r0 < sc_rows else chunk_rows
                        nrows = min(nrows, h - r0)
                        emit_input_rows(r0, nrows)
                        next_row[0] += nrows

                # compute pipeline: super-chunks of sc_rows output rows,
                # one PSUM-bank tile per bank_rows strip (own accum group)
                for s in range(h // sc_rows):
                    # rows needed by SC s (+1 halo) plus one SC of lookahead
                    load_until(min((s + 2) * sc_rows + 1, h))
                    h0 = s * sc_rows
                    pts = [
                        ppool.tile(
                            [M, bank_rows, w], mybir.dt.float32,
                            name="pt", tag="pt",
                        )
                        for _ in range(banks_per_sc)
                    ]
                    for t in range(n_taps):
                        dy, dx = t // 3, t % 3
                        lhsT = wsb[:, t, :]
                        for b in range(banks_per_sc):
                            hb = h0 + b * bank_rows
                            nc.tensor.matmul(
                                pts[b][:],
                                lhsT,
                                xpad[:, hb + dy : hb + dy + bank_rows, dx : dx + w],
                                start=(t == 0),
                                stop=(t == n_taps - 1),
                            )
                    ot = opool.tile([M, sc_rows, w], mybir.dt.float32, name="ot")
                    last_sc = s == h // sc_rows - 1
                    for b in range(banks_per_sc):
                        nc.vector.tensor_scalar_mul(
                            ot[:, b * bank_rows : (b + 1) * bank_rows, :],
                            pts[b][:],
                            1.0 / div,
                        )
                        if last_sc:
                            # per-bank store at the tail: drain->DMA chain of
                            # the final strip instead of the whole super-chunk
                            hb = h0 + b * bank_rows
                            nc.sync.dma_start(
                                out=out_dram[:, hb : hb + bank_rows, :],
                                in_=ot[:, b * bank_rows : (b + 1) * bank_rows, :],
                            )
                    if not last_sc:
                        nc.sync.dma_start(
                            out=out_dram[:, h0 : h0 + sc_rows, :], in_=ot[:]
                        )
                load_until(h)

            for _rep in range(repeats):
                emit_body()

    nc.compile()
    return nc


def make_weight_tensor(weight, nimg=N_IMG, cin=C_IN, cout=C_OUT):
    """Binarize + block-diagonalize: [cout,cin,3,3] f32 -> [nimg*cin, 9, nimg*cout] bf16."""
    n_taps = weight.shape[2] * weight.shape[3]
    wbin = np.where(weight > 0, 1.0, -1.0).astype(np.float32)
    # [co, ci, kh, kw] -> [ci, t, co]
    wt = wbin.reshape(cout, cin, n_taps).transpose(1, 2, 0)
    w9 = np.zeros((nimg * cin, n_taps, nimg * cout), dtype=ml_dtypes.bfloat16)
    for i in range(nimg):
        w9[i * cin : (i + 1) * cin, :, i * cout : (i + 1) * cout] = wt
    return w9


def kernel(x, weight, trace=False, repeats=1, _nc_cache={}):
    """Full-input entry point: x (32,32,256,256) f32, weight (32,32,3,3) f32."""
    x = np.ascontiguousarray(np.asarray(x, dtype=np.float32))
    weight = np.asarray(weight, dtype=np.float32)
    n_batch = x.shape[0]
    per_core = n_batch // N_CORES

    if repeats not in _nc_cache:
        _nc_cache[repeats] = build_conv_kernel(repeats=repeats)
    nc = _nc_cache[repeats]

    w9 = make_weight_tensor(weight)
    P = N_IMG * C_IN
    in_maps = [
        {
            "x": x[i * per_core : (i + 1) * per_core].reshape(P, H, W),
            "w9": w9,
        }
        for i in range(N_CORES)
    ]
    try:
        res = bass_utils.run_bass_kernel_spmd(
            nc, in_maps, core_ids=list(range(N_CORES)), trace=trace
        )
    except ModuleNotFoundError:
        # axon NTFF profiling hook unavailable in this environment
        res = bass_utils.run_bass_kernel_spmd(
            nc, in_maps, core_ids=list(range(N_CORES)), trace=False
        )
    out = np.concatenate(
        [r["out"].reshape(per_core, C_OUT, H, W) for r in res.results], axis=0
    )
    if trace:
        kernel.last_results = res
    return out


# revision 17
# speedup vs baseline: 4757.8822x; 1.0091x over previous
"""Binarized 3x3 conv (BConv2d) on 8 TRN2 NeuronCores.

Problem: x (32, 32, 256, 256) f32, weight (32, 32, 3, 3) f32.
  out = conv2d(x, sign(weight), padding='same') / sqrt(32*9)

Strategy:
  - Data-parallel over batch: core i gets images 4i..4i+3 (no collectives).
  - Per core, pack 4 images x 32 input channels onto the 128 SBUF
    partitions.  Each 3x3 tap becomes ONE K=128, M=128 matmul with a
    block-diagonal (per-image) binarized weight matrix; the 9 taps
    accumulate into PSUM and differ only in the rhs address offset into a
    zero-padded bf16 copy of the input (258x258 per partition).
  - bf16 inputs (weights are exactly +-1 in bf16), fp32 PSUM accumulate.
  - Pipeline: DMA f32 chunk -> SBUF, ScalarE cast+place into padded bf16
    image, 9x4 matmuls per 8-row super-chunk into 4 PSUM banks, VectorE
    scaled drain to SBUF, DMA out.
"""

import numpy as np
import ml_dtypes

import concourse.mybir as mybir
import concourse.tile as tile
from concourse import bacc
from concourse import bass_utils

N_CORES = 8
N_IMG = 4          # images per core
C_IN = 32
C_OUT = 32
K = 3
H = 256
W = 256
DIV = float(np.sqrt(C_IN * K * K))


def build_conv_kernel(
    nimg=N_IMG,
    cin=C_IN,
    cout=C_OUT,
    h=H,
    w=W,
    chunk_rows=8,   # input DMA/cast granularity (rows)
    bank_rows=2,    # output rows per PSUM bank matmul (bank_rows*w <= 512)
    banks_per_sc=4, # PSUM banks per super-chunk
    div=DIV,
    repeats=1,      # execute the whole body N times (for delta-timing)
):
    """Build the per-core Bass graph.  Returns nc (compiled Bacc)."""
    P = nimg * cin
    assert P <= 128
    M = nimg * cout
    assert M <= 128
    assert bank_rows * w <= 512
    sc_rows = bank_rows * banks_per_sc
    assert h % chunk_rows == 0 and h % sc_rows == 0
    hp, wp = h + 2, w + 2
    n_taps = 9

    nc = bacc.Bacc(
        "TRN2", target_bir_lowering=False, debug=False, num_devices=N_CORES
    )
    x_dram = nc.dram_tensor("x", [P, h, w], mybir.dt.float32, kind="ExternalInput")
    w_dram = nc.dram_tensor(
        "w9", [P, n_taps, M], mybir.dt.bfloat16, kind="ExternalInput"
    )
    out_dram = nc.dram_tensor(
        "out", [M, h, w], mybir.dt.float32, kind="ExternalOutput"
    )

    with tile.TileContext(nc) as tc:
        with (
            tc.tile_pool(name="persist", bufs=1) as perpool,
            tc.tile_pool(name="istage", bufs=3) as ipool,
            tc.tile_pool(name="ostage", bufs=3) as opool,
            tc.tile_pool(name="psum", bufs=2 * banks_per_sc, space="PSUM") as ppool,
        ):
            xpad = perpool.tile([P, hp, wp], mybir.dt.bfloat16, name="xpad")
            wsb = perpool.tile([P, n_taps, M], mybir.dt.bfloat16, name="wsb")

            # tap-0 weights first so the first LDWEIGHTS unblocks early
            nc.sync.dma_start(out=wsb[:, 0, :], in_=w_dram[:, 0, :])
            nc.sync.dma_start(out=wsb[:, 1:, :], in_=w_dram[:, 1:, :])
            # zero the padding ring (top/bottom rows, left/right columns)
            nc.vector.memset(xpad[:, 0, :], 0.0)
            nc.vector.memset(xpad[:, hp - 1, :], 0.0)
            nc.vector.memset(xpad[:, :, 0], 0.0)
            nc.vector.memset(xpad[:, :, wp - 1], 0.0)

            def emit_input_rows(r0, nrows):
                st = ipool.tile(
                    [P, chunk_rows, w], mybir.dt.float32, name="st", tag="st"
                )
                nc.sync.dma_start(
                    out=st[:, :nrows, :], in_=x_dram[:, r0 : r0 + nrows, :]
                )
                nc.scalar.copy(
                    out=xpad[:, r0 + 1 : r0 + nrows + 1, 1 : w + 1],
                    in_=st[:, :nrows, :],
                )

            def emit_body():
                # Interleave input chunks with compute super-chunks so DMA
                # lane semaphore windows complete progressively (an
                # up-front input burst couples early sem resets to the
                # last input DMA and stalls the whole pipeline mid-kernel).
                next_row = [0]

                def load_until(row_needed):
                    while next_row[0] < min(row_needed, h):
                        r0 = next_row[0]
                        # small first pieces so the first matmuls start early
                        nrows = 2 if r0 < sc_rows else chunk_rows
                        nrows = min(nrows, h - r0)
                        emit_input_rows(r0, nrows)
                        next_row[0] += nrows

                # compute pipeline: super-chunks of output rows, one
                # PSUM-bank tile per bank_rows strip (own accum group).
                # The last super-chunk tapers (4,2,2 rows) so the final
                # drain->store chain after the last matmul is short.
                plan = []
                r = 0
                while r < h:
                    if h - r > sc_rows:
                        rows = sc_rows
                    elif h - r == sc_rows and sc_rows >= 8:
                        plan += [(r, sc_rows // 2), (r + sc_rows // 2,
                                                     sc_rows // 4)]
                        r += 3 * sc_rows // 4
                        rows = h - r
                    else:
                        rows = h - r
                    plan.append((r, rows))
                    r += rows

                for si, (h0, rows) in enumerate(plan):
                    load_until(min(h0 + rows + sc_rows + 1, h))
                    banks = rows // bank_rows
                    pts = [
                        ppool.tile(
                            [M, bank_rows, w], mybir.dt.float32,
                            name="pt", tag="pt",
                        )
                        for _ in range(banks)
                    ]
                    # first SC: bank-outer so bank 0's accumulation (which
                    # needs only the first 3 input rows) completes first
                    if si == 0:
                        order = [(t, b) for b in range(banks)
                                 for t in range(n_taps)]
                    else:
                        order = [(t, b) for t in range(n_taps)
                                 for b in range(banks)]
                    for t, b in order:
                        dy, dx = t // 3, t % 3
                        hb = h0 + b * bank_rows
                        nc.tensor.matmul(
                            pts[b][:],
                            wsb[:, t, :],
                            xpad[:, hb + dy : hb + dy + bank_rows, dx : dx + w],
                            start=(t == 0),
                            stop=(t == n_taps - 1),
                        )
                    ot = opool.tile(
                        [M, rows, w], mybir.dt.float32, name="ot", tag="ot",
                        padded_shape=[M, sc_rows, w],
                    )
                    for b in range(banks):
                        nc.vector.tensor_scalar_mul(
                            ot[:, b * bank_rows : (b + 1) * bank_rows, :],
                            pts[b][:],
                            1.0 / div,
                        )
                    nc.sync.dma_start(
                        out=out_dram[:, h0 : h0 + rows, :], in_=ot[:]
                    )
                load_until(h)

            for _rep in range(repeats):
                emit_body()

    nc.compile()
    return nc


def make_weight_tensor(weight, nimg=N_IMG, cin=C_IN, cout=C_OUT):
    """Binarize + block-diagonalize: [cout,cin,3,3] f32 -> [nimg*cin, 9, nimg*cout] bf16."""
    n_taps = weight.shape[2] * weight.shape[3]
    wbin = np.where(weight > 0, 1.0, -1.0).astype(np.float32)
    # [co, ci, kh, kw] -> [ci, t, co]
    wt = wbin.reshape(cout, cin, n_taps).transpose(1, 2, 0)
    w9 = np.zeros((nimg * cin, n_taps, nimg * cout), dtype=ml_dtypes.bfloat16)
    for i in range(nimg):
        w9[i * cin : (i + 1) * cin, :, i * cout : (i + 1) * cout] = wt
    return w9


def kernel(x, weight, trace=False, repeats=1, _nc_cache={}):
    """Full-input entry point: x (32,32,256,256) f32, weight (32,32,3,3) f32."""
    x = np.ascontiguousarray(np.asarray(x, dtype=np.float32))
    weight = np.asarray(weight, dtype=np.float32)
    n_batch = x.shape[0]
    per_core = n_batch // N_CORES

    if repeats not in _nc_cache:
        _nc_cache[repeats] = build_conv_kernel(repeats=repeats)
    nc = _nc_cache[repeats]

    w9 = make_weight_tensor(weight)
    P = N_IMG * C_IN
    in_maps = [
        {
            "x": x[i * per_core : (i + 1) * per_core].reshape(P, H, W),
            "w9": w9,
        }
        for i in range(N_CORES)
    ]
    try:
        res = bass_utils.run_bass_kernel_spmd(
            nc, in_maps, core_ids=list(range(N_CORES)), trace=trace
        )
    except ModuleNotFoundError:
        # axon NTFF profiling hook unavailable in this environment
        res = bass_utils.run_bass_kernel_spmd(
            nc, in_maps, core_ids=list(range(N_CORES)), trace=False
        )
    out = np.concatenate(
        [r["out"].reshape(per_core, C_OUT, H, W) for r in res.results], axis=0
    )
    if trace:
        kernel.last_results = res
    return out
